# revision 17
# baseline (speedup 1.0000x reference)
"""Longformer attention Bass kernel for 8 TRN2 NeuronCores (v2.3).

Problem: B=2, H=16, N=2048, D=64, window=256, global positions 0..3.
Sharding: B*H = 32 heads -> 4 heads per core (head-parallel).

Design notes (from trace iterations):
  - The PE only reaches 2.4 GHz (HAM un-throttle) when it is the strict
    pipeline bottleneck; recurring sub-us queue-head waits cold-lock it at
    1.2 GHz.  So the schedule keeps PE work >= ScalarE work per stage.
  - Global QUERIES (rows 0..3 of O) are computed exactly on the host and
    overwrite the device result.
  - Global KEYS are 3 "strip" blocks on device: kg[p] is a block-structured
    [128, 128] weight (head h's 4 global-key columns at 32h), so ONE matmul
    per (pair, strip) computes 2 heads' [4, 512] score strips into disjoint
    partition rows; all 4 heads share one ACTIVATE.  Strip-PV contracts the
    full 128 partitions against vg4[:, h, :] (zero outside rows 32h..32h+4).
  - exp batched: one ACTIVATE per two tightly-packed window chunks from a
    3-bank PSUM slot; two slots ping-pong; pipeline depth 3
    (QK pair g || ACT pair g-1 || PV pair g-2).
  - Window masks: leading triangle DVE-multiplied with a precomputed bf16
    0/1 mask; trailing triangle via GpSimd affine_select.
  - O^T accumulates in 2 rotating PSUM banks (512-query blocks; start=True
    clears a bank at block birth, has_written gives per-element accumulate),
    DVE-copied out per block, DMA'd per block.
  - Startup: qt0 on the sync DGE queue, kt0/vx0 on the scalar DGE queue so
    the first window chunk's inputs land in parallel.
"""

import numpy as np
import ml_dtypes

B, H, N, D = 2, 16, 2048, 64
W = 256
NCORES = 8
HPC = (B * H) // NCORES  # 4 heads per core
NKC = N // 128  # 16 key chunks
SLOTW = 1280
BF16 = ml_dtypes.bfloat16

# ---------------------------------------------------------------------------
# Geometry (shared by numpy model and bass emitter)
# ---------------------------------------------------------------------------


def chunk_qs(kc: int) -> int:
    return 0 if kc <= 1 else 128 * kc - 256


def chunk_w(kc: int) -> int:
    if kc <= 1 or kc == 14:
        return 512
    if kc == 15:
        return 384
    return 640


def chunk_off(kc: int) -> int:
    return 0 if kc % 2 == 0 else chunk_w(kc - 1)


def _split512(a, b):
    out = []
    while a < b:
        m = min((a // 512 + 1) * 512, b)
        out.append((a, m))
        a = m
    return out


def qk_pieces(kc):
    off = chunk_off(kc)
    return _split512(off, off + chunk_w(kc))


def pv_pieces(kc):
    qs = chunk_qs(kc)
    return _split512(qs, qs + chunk_w(kc))


# O^T block b (queries [512b, 512b+512)) first/last contributing chunk.
FIRST_TOUCH = {0: 0, 1: 2, 2: 6, 3: 10}
LAST_TOUCH = {0: 5, 1: 9, 2: 13, 3: 15}
# strip sb in {1,2,3} adds global keys to block sb (block 0 gets them from
# chunk 0 rows 0..3, kept by the m0 mask).


def has_leading(kc):  # mask keep j' >= p at cols [off, off+128)
    return kc >= 2


def has_trailing(kc):  # mask keep j' <= p at cols [off+w-128, off+w)
    return 1 <= kc <= 13


# ---------------------------------------------------------------------------
# Host-side prep / masks
# ---------------------------------------------------------------------------


def build_masks():
    # mask0: chunk0 cols 256..512 (q = 256+j): keep q-k<=256 (j<=k) or k<4
    k = np.arange(128)[:, None]
    j = np.arange(256)[None, :]
    m0 = ((j <= k) | (k <= 3)).astype(np.float32).astype(BF16)
    # maskL: keep j >= p
    p = np.arange(128)[:, None]
    j2 = np.arange(128)[None, :]
    mL = (j2 >= p).astype(np.float32).astype(BF16)
    return m0, mL


def prep_core_inputs(Q, K, V, core):
    """Q/K/V: [B*H, N, D] f32. Returns in_map for one core."""
    h0 = core * HPC
    qt = np.empty((2, 128, N), BF16)
    kt = np.empty((2, 128, N), BF16)
    vx = np.zeros((128, HPC, NKC, 65), BF16)
    kg = np.zeros((2, 128, 128), BF16)
    vg4 = np.zeros((128, HPC, 65), BF16)
    for p in range(2):
        for s in range(2):
            h = h0 + 2 * p + s
            qt[p, 64 * s : 64 * s + 64] = (Q[h].T * np.float32(0.125)).astype(BF16)
            kt[p, 64 * s : 64 * s + 64] = K[h].T.astype(BF16)
            hh = 2 * p + s
            kg[p, 64 * s : 64 * s + 64, 32 * hh : 32 * hh + 4] = K[h][0:4].T.astype(
                BF16
            )
    for i in range(HPC):
        v = np.concatenate(
            [V[h0 + i], np.ones((N, 1), np.float32)], axis=1
        )  # [N, 65]
        vx[:, i] = v.reshape(NKC, 128, 65).transpose(1, 0, 2).astype(BF16)
        vg4[32 * i : 32 * i + 4, i] = v[0:4].astype(BF16)
    m0, mL = build_masks()
    return {
        "qt": qt,
        "kt": kt,
        "vx": vx,
        "kg": kg,
        "vg4": vg4,
        "m0": m0,
        "mL": mL,
    }


def host_global_queries(Q, K, V):
    """Exact fp32 attention for queries 0..3, all heads. Returns [BH,4,D]."""
    Qg = Q[:, 0:4, :]  # [BH, 4, D]
    s = np.einsum("hqd,hkd->hqk", Qg, K) * np.float32(0.125)
    s -= s.max(axis=-1, keepdims=True)
    p = np.exp(s)
    p /= p.sum(axis=-1, keepdims=True)
    return np.einsum("hqk,hkd->hqd", p, V)


def unprep_output(ot_all, Q, K, V):
    """ot_all: [NCORES][HPC, 65, N] f32 -> O [B, H, N, D] f32."""
    out = np.empty((B * H, N, D), np.float32)
    for core in range(NCORES):
        ot = np.array(ot_all[core])
        for i in range(HPC):
            den = ot[i, D]
            den[0:4] = 1.0  # garbage cols, host overwrites below
            out[core * HPC + i] = (ot[i, :D, :] / den).T
    out[:, 0:4, :] = host_global_queries(Q, K, V)
    return out.reshape(B, H, N, D)


# ---------------------------------------------------------------------------
# Numpy model of the device algorithm (geometry validation)
# ---------------------------------------------------------------------------


def numpy_model_core(in_map):
    qt = in_map["qt"].astype(np.float32)
    kt = in_map["kt"].astype(np.float32)
    vx = in_map["vx"].astype(np.float32)
    kg = in_map["kg"].astype(np.float32)
    vg4 = in_map["vg4"].astype(np.float32)
    m0 = in_map["m0"].astype(np.float32)
    mL = in_map["mL"].astype(np.float32)
    ot = np.zeros((HPC, 65, N), np.float32)
    # strips: st_s[sb][128, 512] = kg[0].T@qt[0] + kg[1].T@qt[1]
    pt_s = np.zeros((3, 128, 512), np.float32)
    for sb in (1, 2, 3):
        st = np.zeros((128, 512), np.float32)
        for p in range(2):
            st += kg[p].T @ qt[p, :, 512 * sb : 512 * sb + 512]
        pt_s[sb - 1] = np.exp(st).astype(BF16).astype(np.float32)
    for h in range(HPC):
        p_, s_ = h // 2, h % 2
        qh = qt[p_, 64 * s_ : 64 * s_ + 64]
        kh = kt[p_, 64 * s_ : 64 * s_ + 64]
        for kc in range(NKC):
            qs, w = chunk_qs(kc), chunk_w(kc)
            st = kh[:, 128 * kc : 128 * kc + 128].T @ qh[:, qs : qs + w]
            pt = np.exp(st)
            if kc == 0:
                pt[:, 256:512] *= m0
            if has_leading(kc):
                pt[:, 0:128] *= mL
            if has_trailing(kc):
                tj = np.arange(128)[None, :]
                tp = np.arange(128)[:, None]
                pt[:, w - 128 : w] *= (tj <= tp).astype(np.float32)
            pt = pt.astype(BF16).astype(np.float32)
            ot[h, :, qs : qs + w] += vx[:, h, kc, :].T @ pt
        for sb in (1, 2, 3):
            ot[h, :, 512 * sb : 512 * sb + 512] += vg4[:, h, :].T @ pt_s[sb - 1]
    return ot


# ---------------------------------------------------------------------------
# Bass module
# ---------------------------------------------------------------------------

_CACHED_NC = None


def build_module():
    global _CACHED_NC
    if _CACHED_NC is not None:
        return _CACHED_NC
    from contextlib import ExitStack

    import concourse.bass as bass  # noqa: F401
    import concourse.tile as tile
    from concourse import bacc, mybir

    f32 = mybir.dt.float32
    bf16 = mybir.dt.bfloat16
    EXP = mybir.ActivationFunctionType.Exp
    GE = mybir.AluOpType.is_ge

    nc = bacc.Bacc("TRN2", target_bir_lowering=False, debug=False)
    qt_d = nc.dram_tensor("qt", [2, 128, N], bf16, kind="ExternalInput")
    kt_d = nc.dram_tensor("kt", [2, 128, N], bf16, kind="ExternalInput")
    vx_d = nc.dram_tensor("vx", [128, HPC, NKC, 65], bf16, kind="ExternalInput")
    kg_d = nc.dram_tensor("kg", [2, 128, 128], bf16, kind="ExternalInput")
    vg4_d = nc.dram_tensor("vg4", [128, HPC, 65], bf16, kind="ExternalInput")
    m0_d = nc.dram_tensor("m0", [128, 256], bf16, kind="ExternalInput")
    mL_d = nc.dram_tensor("mL", [128, 128], bf16, kind="ExternalInput")
    ot_d = nc.dram_tensor("ot", [HPC, 65, N], f32, kind="ExternalOutput")

    with tile.TileContext(nc) as tc, ExitStack() as ctx:
        sb = ctx.enter_context(tc.tile_pool(name="sb", bufs=1))
        pt_pool = ctx.enter_context(tc.tile_pool(name="ptp", bufs=4))
        osb_pool = ctx.enter_context(tc.tile_pool(name="osb", bufs=2))
        ps_pool = ctx.enter_context(tc.tile_pool(name="ps", bufs=1, space="PSUM"))

        # --- SBUF-resident tensors; startup DMAs split across the two
        # hardware DGE queues (sync + scalar) so qt0/kt0 land in parallel ---
        m0_sb = sb.tile([128, 256], bf16, tag="m0")
        mL_sb = sb.tile([128, 128], bf16, tag="mL")
        qt_sb = [
            sb.tile([128, N], bf16, tag=f"qt{p}", name=f"qt_sb{p}") for p in range(2)
        ]
        kt_sb = [
            sb.tile([128, N], bf16, tag=f"kt{p}", name=f"kt_sb{p}") for p in range(2)
        ]
        vx_sb = [
            sb.tile([128, NKC, 65], bf16, tag=f"vx{h}", name=f"vx_sb{h}")
            for h in range(HPC)
        ]
        kg_sb = [
            sb.tile([128, 128], bf16, tag=f"kg{p}", name=f"kg_sb{p}")
            for p in range(2)
        ]
        vg4_sb = sb.tile([128, HPC, 65], bf16, tag="vg4")
        pt_s12 = sb.tile([128, 1024], bf16, tag="pts12")
        pt_s3 = sb.tile([128, 512], bf16, tag="pts3")

        nc.sync.dma_start(out=mL_sb[:], in_=mL_d[:])
        nc.sync.dma_start(out=qt_sb[0][:], in_=qt_d[0])
        nc.scalar.dma_start(out=kt_sb[0][:], in_=kt_d[0])
        nc.scalar.dma_start(out=vx_sb[0][:], in_=vx_d[:, 0])
        nc.sync.dma_start(out=m0_sb[:], in_=m0_d[:])
        nc.scalar.dma_start(out=qt_sb[1][:], in_=qt_d[1])
        nc.sync.dma_start(out=kt_sb[1][:], in_=kt_d[1])
        nc.sync.dma_start(out=kg_sb[0][:], in_=kg_d[0])
        nc.sync.dma_start(out=kg_sb[1][:], in_=kg_d[1])
        nc.sync.dma_start(out=vg4_sb[:], in_=vg4_d[:])
        for h in range(1, HPC):
            nc.sync.dma_start(out=vx_sb[h][:], in_=vx_d[:, h])

        # --- PSUM: 2 slots (3 banks each) + 2 rotating O^T banks ---
        slots = [
            ps_pool.tile([128, SLOTW], f32, tag=f"slot{i}", name=f"slot{i}")
            for i in range(2)
        ]
        otb = [
            ps_pool.tile([65, 512], f32, tag=f"otb{i}", name=f"otb{i}")
            for i in range(2)
        ]

        def qh_of(h):
            return qt_sb[h // 2][64 * (h % 2) : 64 * (h % 2) + 64, :]

        def kh_of(h):
            return kt_sb[h // 2][64 * (h % 2) : 64 * (h % 2) + 64, :]

        def emit_qk(h, g):
            slot = slots[g % 2]
            qh, kh = qh_of(h), kh_of(h)
            seen_banks = set()
            for kc in (2 * g, 2 * g + 1):
                qs, off = chunk_qs(kc), chunk_off(kc)
                klhs = kh[:, 128 * kc : 128 * kc + 128]
                for a, b_ in qk_pieces(kc):
                    bank = a // 512
                    first = bank not in seen_banks
                    seen_banks.add(bank)
                    nc.tensor.matmul(
                        slot[:, a:b_],
                        klhs,
                        qh[:, qs + (a - off) : qs + (b_ - off)],
                        start=first,
                        stop=True,
                        skip_group_check=True,
                    )

        def emit_act_masks(h, g):
            slot = slots[g % 2]
            pw = chunk_w(2 * g) + chunk_w(2 * g + 1)
            pt = pt_pool.tile([128, SLOTW], bf16, tag="pt")
            nc.scalar.activation(pt[:, 0:pw], slot[:, 0:pw], EXP)
            for kc in (2 * g, 2 * g + 1):
                off, w = chunk_off(kc), chunk_w(kc)
                if kc == 0:
                    nc.vector.tensor_mul(pt[:, 256:512], pt[:, 256:512], m0_sb[:])
                if has_leading(kc):
                    nc.vector.tensor_mul(
                        pt[:, off : off + 128], pt[:, off : off + 128], mL_sb[:]
                    )
                if has_trailing(kc):
                    c0 = off + w - 128
                    nc.gpsimd.affine_select(
                        pt[:, c0 : c0 + 128],
                        pt[:, c0 : c0 + 128],
                        pattern=[[-1, 128]],
                        base=0,
                        channel_multiplier=1,
                        compare_op=GE,
                        fill=0.0,
                    )
            return pt

        def emit_strip_pv(h, sb_):
            src = pt_s3 if sb_ == 3 else pt_s12
            cols = slice(0, 512) if sb_ != 2 else slice(512, 1024)
            nc.tensor.matmul(
                otb[sb_ % 2][:, 0:512],
                vg4_sb[:, h, :],
                src[:, cols],
                start=False,
                stop=False,
                skip_group_check=True,
            )

        def emit_pv(h, g, pt, ot_sb):
            for kc in (2 * g, 2 * g + 1):
                qs, off = chunk_qs(kc), chunk_off(kc)
                if kc == 9 or kc == 15:
                    emit_strip_pv(h, {9: 1, 15: 3}[kc])
                if kc == 13:
                    emit_strip_pv(h, 2)
                for a, b_ in pv_pieces(kc):
                    blk = a // 512
                    nc.tensor.matmul(
                        otb[blk % 2][:, a - 512 * blk : b_ - 512 * blk],
                        vx_sb[h][:, kc, :],
                        pt[:, off + (a - qs) : off + (b_ - qs)],
                        start=(kc == FIRST_TOUCH[blk]),
                        stop=(kc == LAST_TOUCH[blk]),
                        skip_group_check=True,
                    )
                for blk, last in LAST_TOUCH.items():
                    if kc != last:
                        continue
                    nc.vector.tensor_copy(
                        out=ot_sb[:, 512 * blk : 512 * blk + 512],
                        in_=otb[blk % 2][:, 0:512],
                    )
                    nc.sync.dma_start(
                        out=ot_d[h][:, 512 * blk : 512 * blk + 512],
                        in_=ot_sb[:, 512 * blk : 512 * blk + 512],
                    )

        def emit_strips():
            # strips 1,2 in slots[0] banks 0,1; strip 3 in slots[1] bank 0.
            for sb_ in (1, 2, 3):
                slot = slots[0] if sb_ != 3 else slots[1]
                cols = slice(0, 512) if sb_ != 2 else slice(512, 1024)
                for p_ in range(2):
                    nc.tensor.matmul(
                        slot[:, cols],
                        kg_sb[p_][:, 0:128],
                        qt_sb[p_][:, 512 * sb_ : 512 * sb_ + 512],
                        start=(p_ == 0),
                        stop=(p_ == 1),
                        skip_group_check=True,
                    )
            nc.scalar.activation(pt_s12[:, 0:1024], slots[0][:, 0:1024], EXP)
            nc.scalar.activation(pt_s3[:, 0:512], slots[1][:, 0:512], EXP)

        # --- PE warmup on a memset scratch tile: starts right after the
        # engine preamble (no DMA dependency), ends before qt0/kt0 land, so
        # the HAM is un-throttled when the first real QK issues.  slots[1]
        # bank 2 is first used at pair 3; its first real piece is
        # start=True so the garbage is harmless. ---
        wu_sb = sb.tile([128, 128], bf16, tag="wu")
        nc.vector.memset(wu_sb[:], 0.0)
        for wu in range(52):
            nc.tensor.matmul(
                slots[1][:, 1024:1152],
                wu_sb[:],
                wu_sb[:],
                start=(wu == 0),
                stop=True,
                skip_group_check=True,
            )

        # --- main emission: depth-3 pipeline, flattened across heads so
        # the pipe never drains at head boundaries ---
        pts = {}
        ot_sbs = {}
        NG = 8 * HPC
        for G in range(NG + 2):
            if G < NG:
                emit_qk(G // 8, G % 8)
                pts[G] = emit_act_masks(G // 8, G % 8)
            if G >= 2:
                Gp = G - 2
                hp = Gp // 8
                if hp not in ot_sbs:
                    ot_sbs[hp] = osb_pool.tile(
                        [65, N], f32, tag="otsb", name=f"ot_sb{hp}"
                    )
                emit_pv(hp, Gp % 8, pts.pop(Gp), ot_sbs[hp])
            if G == 2:
                emit_strips()

    nc.compile()
    _CACHED_NC = nc
    return nc


# ---------------------------------------------------------------------------
# Entry points
# ---------------------------------------------------------------------------


def run(inputs, trace=False, trace_kwargs=None):
    from concourse import bass_utils

    Q = np.asarray(inputs["Q"], np.float32).reshape(B * H, N, D)
    K = np.asarray(inputs["K"], np.float32).reshape(B * H, N, D)
    V = np.asarray(inputs["V"], np.float32).reshape(B * H, N, D)
    in_maps = [prep_core_inputs(Q, K, V, c) for c in range(NCORES)]
    nc = build_module()
    res = bass_utils.run_bass_kernel_spmd(
        nc,
        in_maps,
        core_ids=list(range(NCORES)),
        trace=trace,
        **(trace_kwargs or {}),
    )
    ot_all = [res.results[c]["ot"] for c in range(NCORES)]
    return unprep_output(ot_all, Q, K, V), res


def kernel(**inputs) -> np.ndarray:
    out, _ = run(inputs, trace=False)
    return out
